# revision 43
# baseline (speedup 1.0000x reference)
"""DiT attention (B=2, S=2048, DIM=1024, H=16, D=64) on 8 TRN2 NeuronCores.

Sharding: data-parallel over B (2) x tensor-parallel over head groups (4),
so each core owns (one batch, 4 heads). The out-projection is computed as
per-core partials over the 256 e-channels each core owns; the host sums the
4 partials per batch and adds out_b (+ the host-folded V-bias term).

PE and ScalarE are nearly balanced (~150us each of streaming demand), so
the kernel runs a continuous exp stream (128 x [128,2x512] ACTIVATEs) and
packs all other matmul work into its shadow:
  - score matmuls are head-PAIRS on 64-row PE tiles (0,0)/(64,0) so both
    heads' scores stream in one 512-cycle pass
  - pair-ahead emission: score pair kt+1 is emitted BEFORE slot kt's work,
    so the strict-FIFO PE never queues an exp's producer behind a stalled
    slot item
  - a static slot schedule places K/Q/V projections, the previous windows'
    AV accumulation, out-projection and RoPE between score pairs, with
    per-window PE budgets kept under the 18.3us exp budget; AV drains
    aggressively in W6/W7 so only ~3 quarters + norms + 4 out-proj tiles
    remain after the last exp
  - biases: K-et1 bias dropped entirely (q.bk is constant over keys ->
    softmax-invariant), Q and K-et0 biases ride the PSUM->SBUF drain as
    per-partition tensor_scalar adds, V bias is folded into the host sum
  - the lead-in interleaves K-et1 and Q-et1 chunk-0 matmuls per hT slab
    (two open PSUM groups) so projections pipeline with DMA arrival;
    first exp fires at ~20us
  - DMA: sync/gpsimd rings stream hT (chunk halves, chunk0 split further
    into slab-pair quarters); the scalar ring carries only the two tiny
    bias tensors so ACTIVATEs never sit behind a descriptor write
  - softmax normalization stages numerator/denominator rows out of PSUM
    first (frees the AV accumulator bank ~3us earlier), then
    reciprocal + gpsimd partition-broadcast + multiply, two heads
    interleaved
  - V head slots are 66 cols (64 V + ones column for the denominator row
    + pad), avoiding the 128-col layout's ~7us memset
  - warmup matmuls bridge the DMA lead-in and the tail norm chain so the
    PE HAM clock gate stays at 2.4 GHz throughout
"""

import collections

import numpy as np
import ml_dtypes

import concourse.bacc as bacc
import concourse.bass as bass
import concourse.mybir as mybir
import concourse.tile as tile
from concourse.bass_utils import run_bass_kernel_spmd

B, S, DIM, H, D = 2, 2048, 1024, 16, 64
NCORES = 8
GROUPS = 4     # head groups (tensor parallel)
HPG = 4        # heads per group
E = HPG * D    # 256 e-channels per core per projection
P = 128        # partitions
SC = 512       # free-dim chunk for matmuls
NKT = S // P   # 16 key tiles
NQC = S // SC  # 4 query chunks
NST = S // P   # 16 s tiles
NT = DIM // P  # 8 contraction slabs
BF = mybir.dt.bfloat16
F32 = mybir.dt.float32

_BF_NP = ml_dtypes.bfloat16


def _build_nc():
    nc = bacc.Bacc(None, target_bir_lowering=False)

    # host-relaid inputs (see _shard_inputs): partition-major, so every
    # per-partition row below is contiguous in DRAM.  wk/wq are et-major
    # ([P, et, kt, 128]) so the critical et1 half is one contiguous DMA.
    hT_d = nc.declare_dram_parameter("hTc", [NQC * P, NT * SC], BF,
                                     isOutput=False)
    wk_d = nc.declare_dram_parameter("wkT", [P, 2 * NT * P], BF,
                                     isOutput=False)
    wq_d = nc.declare_dram_parameter("wqT", [P, 2 * NT * P], BF,
                                     isOutput=False)
    wv_d = nc.declare_dram_parameter("wvT", [P, NT * E], BF, isOutput=False)
    wout_d = nc.declare_dram_parameter("woutT", [P, (E // P) * DIM], BF,
                                       isOutput=False)
    # Q bias as a per-partition column per et; K bias only for et0 (heads
    # 0,1 — head0's RoPE makes it non-invariant there).  K et1 bias adds
    # q.bk, constant over keys -> softmax-invariant -> dropped.  V bias is
    # folded into the host-side output sum (out_w @ b_v).
    qb_d = nc.declare_dram_parameter("qb", [P, 2], F32, isOutput=False)
    kb0_d = nc.declare_dram_parameter("kb0", [P, 1], F32, isOutput=False)
    cos_d = nc.declare_dram_parameter("cos_t", [D, S], BF, isOutput=False)
    sin_d = nc.declare_dram_parameter("sin_t", [D, S], BF, isOutput=False)
    perm_d = nc.declare_dram_parameter("perm", [D, D], BF, isOutput=False)
    y_d = nc.declare_dram_parameter("y", [S, DIM], BF, isOutput=True)

    with tile.TileContext(nc) as tc:
        import contextlib
        with contextlib.ExitStack() as ctx:
            consts = ctx.enter_context(tc.tile_pool(name="consts", bufs=1))
            ptpool = ctx.enter_context(tc.tile_pool(name="ptpool", bufs=2))
            work = ctx.enter_context(tc.tile_pool(name="work", bufs=2))
            psum = ctx.enter_context(
                tc.tile_pool(name="psum", bufs=2, space="PSUM"))

            # ---- persistent SBUF tensors ----
            # hT chunk-major: [p, chunk, slab, s-in-chunk]
            hT_sb = consts.tile([P, NQC, NT, SC], BF, name="hT_sb")
            wk_sb = consts.tile([P, 2, NT, P], BF, name="wk_sb")
            wq_sb = consts.tile([P, 2, NT, P], BF, name="wq_sb")
            wv_sb = consts.tile([P, NT, E], BF, name="wv_sb")
            wout_sb = consts.tile([P, E // P, DIM], BF, name="wout_sb")
            qb_sb = consts.tile([P, 2], F32, name="qb_sb")
            kb0_sb = consts.tile([P, 1], F32, name="kb0_sb")
            cos_sb = consts.tile([D, S], BF, name="cos_sb")
            sin_sb = consts.tile([D, S], BF, name="sin_sb")
            perm_sb = consts.tile([D, D], BF, name="perm_sb")
            ones_sb = consts.tile([1, SC], BF, name="ones_sb")
            dummy_sb = consts.tile([P, 2 * P], BF, name="dummy_sb")
            # head slots are 66 cols: V in 0:64, ones at 64 (denominator
            # row), zero pad at 65 (even stride).  Narrow slots avoid the
            # ~7us full-tile memset the 128-col layout needed.
            VW = 66
            V_sb = consts.tile([P, NKT, HPG, VW], BF, name="V_sb")
            QT_sb = consts.tile([P, E // P, S], BF, name="QT_sb")
            KT_sb = consts.tile([P, E // P, S], BF, name="KT_sb")
            OT_sb = consts.tile([P, E // P, S], BF, name="OT_sb")
            q0r = consts.tile([D, S], BF, name="q0r")
            k0r = consts.tile([D, S], BF, name="k0r")

            # ---- DMA loads ----
            # Three DMA queues (sync/gpsimd/scalar data rings, ~85-100GB/s
            # each), loaded in need-order.  sync+gpsimd stream hT (the bulk,
            # 4MB) as chunk halves; the scalar ring carries all weights +
            # trig, descriptors written up-front so the data drains
            # asynchronously while ACTIVATEs run.  Gate for the first exp:
            # wk-et1 + wq-et1 (first on the scalar ring) and hT chunk0.
            half = NT * SC // 2          # slab-aligned halves
            hT_flat = hT_sb[:, :, :, :].rearrange("p c t s -> p c (t s)")
            wk_flat = wk_sb[:, :, :, :].rearrange("p e t c -> p (e t c)")
            wq_flat = wq_sb[:, :, :, :].rearrange("p e t c -> p (e t c)")
            ET = NT * P                  # cols per et half

            def htdma(q, scn, c0, c1):
                q.dma_start(
                    out=hT_flat[:, scn, c0:c1],
                    in_=hT_d.ap()[scn * P:(scn + 1) * P, c0:c1])

            # sync ring: wk-et1 then hT c0 slab-pair quarters (K-proj
            # matmuls pipeline with slab arrival), then the rest of h0
            nc.sync.dma_start(out=wk_flat[:, ET:2 * ET],
                              in_=wk_d.ap()[:, ET:2 * ET])
            htdma(nc.sync, 0, 0, SC * 2)
            htdma(nc.sync, 0, SC * 2, half)
            htdma(nc.sync, 1, 0, half)
            htdma(nc.sync, 2, 0, half)
            nc.sync.dma_start(out=wk_flat[:, 0:ET], in_=wk_d.ap()[:, 0:ET])
            nc.sync.dma_start(out=perm_sb[:, :], in_=perm_d.ap())
            nc.sync.dma_start(out=cos_sb[:, :], in_=cos_d.ap())
            htdma(nc.sync, 3, 0, half)
            nc.sync.dma_start(
                out=wv_sb[:, :, :].rearrange("p t e -> p (t e)"),
                in_=wv_d.ap())
            # gpsimd ring: wq-et1 then hT c0 h1 quarters + wq-et0 + wout
            nc.gpsimd.dma_start(out=wq_flat[:, ET:2 * ET],
                                in_=wq_d.ap()[:, ET:2 * ET])
            htdma(nc.gpsimd, 0, half, half + SC * 2)
            htdma(nc.gpsimd, 0, half + SC * 2, 2 * half)
            htdma(nc.gpsimd, 1, half, 2 * half)
            htdma(nc.gpsimd, 2, half, 2 * half)
            nc.gpsimd.dma_start(out=wq_flat[:, 0:ET], in_=wq_d.ap()[:, 0:ET])
            nc.gpsimd.dma_start(out=sin_sb[:, :], in_=sin_d.ap())
            htdma(nc.gpsimd, 3, half, 2 * half)
            nc.gpsimd.dma_start(
                out=wout_sb[:, :, :].rearrange("p t o -> p (t o)"),
                in_=wout_d.ap())
            # scalar ring: only the tiny biases, so its HBM pull never
            # competes with the critical path and exps never queue behind
            # a descriptor write
            nc.scalar.dma_start(out=qb_sb[:, :], in_=qb_d.ap())
            nc.scalar.dma_start(out=kb0_sb[:, :], in_=kb0_d.ap())

            nc.vector.memset(dummy_sb[:, :], 0.5)
            nc.vector.memset(ones_sb[:, :], 1.0)
            nc.vector.memset(V_sb[:, :, :, D:D + 1], 1.0)
            nc.vector.memset(V_sb[:, :, :, D + 1:D + 2], 0.0)

            # ---- PE warmup: real 128-row matmuls spanning the DMA lead-in
            # so the HAM clock gate is at 2.4 GHz when projections start ----
            for w in range(28):
                warm_ps = psum.tile([P, 2 * P], F32, name="warm_ps",
                                    tag="mm512")
                nc.tensor.matmul(
                    out=warm_ps[:, :], lhsT=dummy_sb[:, 0:P],
                    rhs=dummy_sb[:, :], start=True, stop=True)

            # ---------- emitters ----------
            pt_map = {}   # (window, half) -> PT half tile
            o_map = {}    # (window, j)    -> AV psum accumulator
            qk_map = {}   # (which, et, scn) -> projection psum accumulator

            def emit_kq_half(which, et, scn, hf):
                # hf 0/1 of a K/Q projection; the drain lands with hf 1.
                # Q bias rides the PSUM->SBUF cast as a per-partition
                # tensor_scalar add (free); K et0 keeps its K=1 bias matmul
                # (head0's RoPE breaks softmax-invariance); K et1 has none.
                dst = QT_sb if which == 0 else KT_sb
                wsrc = wq_sb if which == 0 else wk_sb
                if hf == 0:
                    qk_map[(which, et, scn)] = psum.tile(
                        [P, SC], F32, name="qk_ps", tag="mm512")
                qk_ps = qk_map[(which, et, scn)]
                for kt in range(hf * NT // 2, (hf + 1) * NT // 2):
                    nc.tensor.matmul(
                        out=qk_ps[:, :],
                        lhsT=wsrc[:, et, kt, :],
                        rhs=hT_sb[:, scn, kt, :],
                        start=(kt == 0), stop=False,
                        skip_group_check=True)
                if hf == 1:
                    out_ap = dst[:, et, scn * SC:(scn + 1) * SC]
                    if which == 0:
                        nc.vector.tensor_scalar_add(
                            out_ap, qk_ps[:, :], qb_sb[:, et:et + 1])
                    elif et == 0:
                        nc.vector.tensor_scalar_add(
                            out_ap, qk_ps[:, :], kb0_sb[:, 0:1])
                    else:
                        nc.vector.tensor_copy(out=out_ap, in_=qk_ps[:, :])

            def emit_kq_proj(which, et, scn):
                emit_kq_half(which, et, scn, 0)
                emit_kq_half(which, et, scn, 1)

            def emit_rope(which, scn):
                # head-0 RoPE for chunk scn: dst = src*cos + swap(src)*sin_s
                src = QT_sb if which == 0 else KT_sb
                dst = q0r if which == 0 else k0r
                tmp = work.tile([D, SC], BF, name="rtmp", tag="rtmp", bufs=2)
                s_sl = slice(scn * SC, (scn + 1) * SC)
                nc.vector.tensor_mul(
                    out=tmp[:, :], in0=src[0:D, 0, s_sl],
                    in1=cos_sb[:, s_sl])
                sw_ps = psum.tile([D, SC], F32, name="sw_ps", tag="mm512")
                nc.tensor.matmul(
                    out=sw_ps[:, :], lhsT=perm_sb[:, :],
                    rhs=src[0:D, 0, s_sl], start=True, stop=True)
                nc.vector.tensor_mul(
                    out=dst[:, s_sl], in0=sw_ps[:, :], in1=sin_sb[:, s_sl])
                nc.vector.tensor_add(
                    out=dst[:, s_sl], in0=dst[:, s_sl], in1=tmp[:, :])

            def emit_v_proj(st):
                # V bias is folded into the host-side output sum
                v_ps = psum.tile([P, E], F32, name="v_ps", tag="mm512")
                for kt in range(NT):
                    nc.tensor.matmul(
                        out=v_ps[:, :],
                        lhsT=hT_sb[:, st // 4, kt,
                                   (st % 4) * P:(st % 4 + 1) * P],
                        rhs=wv_sb[:, kt, :],
                        start=(kt == 0), stop=(kt == NT - 1))
                nc.vector.tensor_copy(
                    out=V_sb[:, st, :, 0:D],
                    in_=v_ps[:, :].rearrange("p (h c) -> p h c", h=HPG))

            def head_qk(h):
                if h == 0:
                    return q0r[:, :], k0r[:, :]
                po = (h % 2) * D
                return (QT_sb[po:po + D, h // 2, :],
                        KT_sb[po:po + D, h // 2, :])

            # window -> (qc, head pair): even windows run heads (2,3)
            # first (no RoPE -> cos/sin off the load critical path)
            def win_heads(w):
                qc, p = divmod(w, 2)
                return qc, (2, 3) if p == 0 else (0, 1)

            def av_part(w, j, kt_lo, kt_hi):
                # partial AV accumulation for head j of window w's pair
                qc, heads = win_heads(w)
                h = heads[j]

                def f():
                    if (w, j) not in o_map:
                        o_map[(w, j)] = psum.tile([P, SC], F32, name="o_ps",
                                                  tag="o_ps")
                    o_ps = o_map[(w, j)]
                    for kt in range(kt_lo, kt_hi):
                        nc.tensor.matmul(
                            out=o_ps[0:VW, :],
                            lhsT=V_sb[:, kt, h, :],
                            rhs=pt_map[(w, kt // 8)][:, kt % 8, j, :],
                            start=(kt == 0), stop=(kt == NKT - 1),
                            skip_group_check=True)
                return f

            def norm(w, j, tail=False):
                # normalize head j of window w: OT = (O^T_unnorm) * (1/den)
                qc, heads = win_heads(w)
                h = heads[j]
                q_sl = slice(qc * SC, (qc + 1) * SC)

                def f():
                    o_ps = o_map[(w, j)]
                    po = (h % 2) * D
                    # custom-DVE bitwise ops give garbage on a PSUM read path
                    # (HW-only; sim is clean) — stage the row in SBUF first
                    denr = work.tile([1, SC], F32, name="denr", tag="denr",
                                     bufs=2)
                    if tail:
                        # ScalarE is idle after the last exp — use it
                        nc.scalar.copy(out=denr[:, :], in_=o_ps[D:D + 1, :])
                    else:
                        nc.vector.tensor_copy(out=denr[:, :], in_=o_ps[D:D + 1, :])
                    rcp = work.tile([1, SC], F32, name="rcp", tag="rcp",
                                    bufs=2)
                    nc.vector.reciprocal_approx_fast(
                        out=rcp[:, :], in_=denr[:, :])
                    # partition-broadcast 1/denom on the idle GpSimd
                    # engine (no PE matmul, no PSUM bank)
                    rbc = work.tile([D, SC], F32, name="rbc", tag="rbc",
                                    bufs=2)
                    nc.gpsimd.partition_broadcast(rbc[:, :], rcp[:, :])
                    # multiply straight out of PSUM — skips the otu staging
                    # copy, shortening the norm chain by ~0.7us
                    nc.vector.tensor_mul(
                        out=OT_sb[po:po + D, h // 2, q_sl],
                        in0=o_ps[0:D, :], in1=rbc[:, :])
                return f

            def norm2(w, tail=False):
                # both heads' norms with the DVE ops interleaved so the
                # reciprocal of head1 runs while head0's gpsimd broadcast
                # is in flight (the serial 2x5-op chain was starving the
                # PE long enough to drop the HAM clock to 1.2GHz)
                qc, heads = win_heads(w)
                q_sl = slice(qc * SC, (qc + 1) * SC)

                def f():
                    ot, rb = [], []
                    for j in (0, 1):
                        o_ps = o_map[(w, j)]
                        # stage numerator + denominator rows out of PSUM
                        # first so o_ps frees after ~1.4us, not after the
                        # full reciprocal/broadcast/multiply chain — the
                        # next window's AV accumulator reuses these banks
                        otu = work.tile([D, SC], BF, name="otu", tag="otu",
                                        bufs=2)
                        denr = work.tile([1, SC], F32, name="denr",
                                         tag="denr", bufs=2)
                        if tail:
                            nc.scalar.copy(out=otu[:, :], in_=o_ps[0:D, :])
                            nc.vector.tensor_copy(out=denr[:, :],
                                                  in_=o_ps[D:D + 1, :])
                        else:
                            nc.vector.tensor_copy(out=otu[:, :],
                                                  in_=o_ps[0:D, :])
                            nc.vector.tensor_copy(out=denr[:, :],
                                                  in_=o_ps[D:D + 1, :])
                        rcp = work.tile([1, SC], F32, name="rcp", tag="rcp",
                                        bufs=2)
                        nc.vector.reciprocal_approx_fast(
                            out=rcp[:, :], in_=denr[:, :])
                        rbc = work.tile([D, SC], F32, name="rbc", tag="rbc",
                                        bufs=2)
                        nc.gpsimd.partition_broadcast(rbc[:, :], rcp[:, :])
                        ot.append(otu); rb.append(rbc)
                    for j in (0, 1):
                        h = heads[j]
                        po = (h % 2) * D
                        nc.vector.tensor_mul(
                            out=OT_sb[po:po + D, h // 2, q_sl],
                            in0=ot[j][:, :], in1=rb[j][:, :])
                return f

            def out_proj_st(st, yq=None, tail=False):
                def f():
                    y_sb = work.tile([P, DIM], BF, name="y_sb", tag="y_sb")
                    ets = (1, 0) if tail else (0, 1)
                    for oc in range(DIM // SC):
                        tag = ("o_ps" if (tail and oc == 1) else "mm512")
                        y_ps = psum.tile([P, SC], F32, name="y_ps", tag=tag)
                        for i, et in enumerate(ets):
                            nc.tensor.matmul(
                                out=y_ps[:, :],
                                lhsT=OT_sb[:, et, st * P:(st + 1) * P],
                                rhs=wout_sb[:, et, oc * SC:(oc + 1) * SC],
                                start=(i == 0), stop=(i == E // P - 1))
                        # GpSimd has no PSUM port on trn2; in the tail the
                        # idle ScalarE drains half the banks and each half
                        # ships as its own DMA so transfers overlap
                        if tail and oc == 1:
                            nc.scalar.copy(
                                out=y_sb[:, oc * SC:(oc + 1) * SC],
                                in_=y_ps[:, :])
                        else:
                            nc.vector.tensor_copy(
                                out=y_sb[:, oc * SC:(oc + 1) * SC],
                                in_=y_ps[:, :])
                        if tail:
                            yq[oc].dma_start(
                                out=y_d.ap()[st * P:(st + 1) * P,
                                             oc * SC:(oc + 1) * SC],
                                in_=y_sb[:, oc * SC:(oc + 1) * SC])
                    if not tail:
                        (yq or nc.sync).dma_start(
                            out=y_d.ap()[st * P:(st + 1) * P, :],
                            in_=y_sb[:, :])
                return f

            def kqh(which, et, scn, hf):
                return lambda: emit_kq_half(which, et, scn, hf)

            def kq(which, et, scn):
                return lambda: emit_kq_proj(which, et, scn)

            def rope(which, scn):
                return lambda: emit_rope(which, scn)

            def vp(st):
                return lambda: emit_v_proj(st)

            # ---------- static slot schedule ----------
            # sched[(w, s)] emits after score-pair s of window w; slot -1
            # runs before the window's first scores.  Producers always sit
            # at an earlier emission point than their consumers.
            sched = collections.defaultdict(list)

            # W0 (qc0, heads 2,3): K et0 c0 + k0r ropes moved here from the
            # lead-in (shorter path to the first exp).  K et1 chunk c sits
            # at the latest slot before kt=4c so DMA-arrival stalls don't
            # block earlier score pairs; ropes go late (results are
            # W1-needed, cos/sin arrive ~22-24us).
            sched[(0, 0)].append(kq(1, 1, 1))    # KT et1 c1 before kt4
            sched[(0, 3)].append(kqh(1, 1, 2, 0))   # before kt8
            sched[(0, 4)].append(kqh(1, 1, 2, 1))
            sched[(0, 5)].append(kqh(1, 0, 0, 0))
            sched[(0, 6)].append(kqh(1, 0, 0, 1))
            sched[(0, 7)].append(kq(0, 0, 0))    # Q et0 c0 for W1
            sched[(0, 8)].append(kqh(1, 1, 3, 0))   # before kt12
            sched[(0, 9)].append(kqh(1, 1, 3, 1))
            sched[(0, 10)].append(rope(1, 0))    # k0r chunk0 for W1
            sched[(0, 11)].append(kq(1, 0, 1))
            sched[(0, 12)].append(rope(1, 1))
            sched[(0, 13)].append(kq(1, 0, 2))
            sched[(0, 13)].append(kqh(1, 0, 3, 0))
            sched[(0, 14)].append(rope(0, 0))    # q0r chunk0 for W1
            sched[(0, 14)].append(rope(1, 2))
            sched[(0, 15)].append(kqh(1, 0, 3, 1))
            sched[(0, 15)].append(rope(1, 3))

            # W1 (qc0, heads 0,1): V projection starts (wv mid-W1)
            sched[(1, 0)].append(kqh(0, 1, 1, 0))
            sched[(1, 1)].append(kqh(0, 1, 1, 1))  # QT et1 c1 for W2
            for i in range(9):
                sched[(1, 2 + i)].append(vp(i))
            sched[(1, 11)].append(av_part(0, 0, 0, 4))
            sched[(1, 12)].append(av_part(0, 0, 4, 8))
            sched[(1, 13)].append(av_part(0, 1, 0, 4))
            sched[(1, 14)].append(av_part(0, 1, 4, 8))

            # W2 (qc1, heads 2,3): rest of V, finish AV(0), start AV(1)
            for i in range(7):
                sched[(2, i)].append(vp(9 + i))
            sched[(2, 7)].append(av_part(0, 0, 8, 12))
            sched[(2, 7)].append(kqh(0, 0, 1, 0))
            sched[(2, 8)].append(av_part(0, 1, 8, 12))
            sched[(2, 9)].append(av_part(0, 0, 12, 16))
            sched[(2, 9)].append(kqh(0, 0, 1, 1))
            sched[(2, 10)].append(av_part(0, 1, 12, 16))
            sched[(2, 11)].append(rope(0, 1))    # q0r chunk1 for W3
            sched[(2, 11)].append(norm2(0))
            sched[(2, 14)].append(av_part(1, 0, 0, 4))
            sched[(2, 15)].append(av_part(1, 1, 0, 4))

            # W3 (qc1, heads 0,1): finish AV(1), out-proj st0-1, start AV(2)
            sched[(3, 0)].append(av_part(1, 0, 4, 8))
            sched[(3, 1)].append(av_part(1, 1, 4, 8))
            sched[(3, 2)].append(av_part(1, 0, 8, 12))
            sched[(3, 3)].append(av_part(1, 1, 8, 12))
            sched[(3, 4)].append(av_part(1, 0, 12, 16))
            sched[(3, 5)].append(av_part(1, 1, 12, 16))
            sched[(3, 6)].append(kqh(0, 1, 2, 0))
            sched[(3, 7)].append(kqh(0, 1, 2, 1))  # QT et1 c2 for W4
            sched[(3, 7)].append(norm2(1))
            sched[(3, 10)].append(av_part(2, 0, 0, 4))
            sched[(3, 11)].append(av_part(2, 1, 0, 4))
            sched[(3, 12)].append(out_proj_st(0))
            sched[(3, 13)].append(out_proj_st(1))
            sched[(3, 13)].append(av_part(2, 0, 4, 8))
            sched[(3, 14)].append(av_part(2, 1, 4, 8))

            # W4 (qc2, heads 2,3): finish AV(2), st2-3, AV(3) 6 quarters
            sched[(4, 0)].append(av_part(2, 0, 8, 12))
            sched[(4, 1)].append(av_part(2, 1, 8, 12))
            sched[(4, 2)].append(av_part(2, 0, 12, 16))
            sched[(4, 3)].append(av_part(2, 1, 12, 16))
            sched[(4, 4)].append(kqh(0, 0, 2, 0))
            sched[(4, 5)].append(kqh(0, 0, 2, 1))
            sched[(4, 5)].append(norm2(2))
            sched[(4, 6)].append(rope(0, 2))     # q0r chunk2 for W5
            sched[(4, 8)].append(out_proj_st(2))
            sched[(4, 9)].append(out_proj_st(3))
            sched[(4, 10)].append(av_part(3, 0, 0, 4))
            sched[(4, 11)].append(av_part(3, 1, 0, 4))
            sched[(4, 12)].append(av_part(3, 0, 4, 8))
            sched[(4, 13)].append(av_part(3, 1, 4, 8))
            sched[(4, 14)].append(av_part(3, 0, 8, 12))
            sched[(4, 15)].append(av_part(3, 1, 8, 12))

            # W5 (qc2, heads 0,1): finish AV(3), st4-7, AV(4) 6 quarters
            sched[(5, 0)].append(av_part(3, 0, 12, 16))
            sched[(5, 1)].append(av_part(3, 1, 12, 16))
            sched[(5, 2)].append(kqh(0, 1, 3, 0))
            sched[(5, 3)].append(kqh(0, 1, 3, 1))  # QT et1 c3 for W6
            sched[(5, 3)].append(norm2(3))
            sched[(5, 5)].append(av_part(4, 0, 0, 4))
            sched[(5, 6)].append(av_part(4, 1, 0, 4))
            sched[(5, 7)].append(out_proj_st(4))
            sched[(5, 8)].append(out_proj_st(5))
            sched[(5, 9)].append(av_part(4, 0, 4, 8))
            sched[(5, 10)].append(out_proj_st(6))
            sched[(5, 11)].append(out_proj_st(7))
            sched[(5, 12)].append(av_part(4, 1, 4, 8))
            sched[(5, 13)].append(av_part(4, 0, 8, 12))
            sched[(5, 14)].append(av_part(4, 1, 8, 12))

            # W6 (qc3, heads 2,3): AV(5) entirely here; K et0 c3 + q0r c3
            # early so W7's first score pair is never rope/DVE-gated
            sched[(6, 0)].append(av_part(4, 0, 12, 16))
            sched[(6, 1)].append(av_part(4, 1, 12, 16))
            sched[(6, 2)].append(norm2(4))
            sched[(6, 2)].append(kqh(0, 0, 3, 0))
            sched[(6, 3)].append(av_part(5, 0, 0, 4))
            sched[(6, 4)].append(kqh(0, 0, 3, 1))
            sched[(6, 5)].append(av_part(5, 1, 0, 4))
            sched[(6, 6)].append(rope(0, 3))     # q0r chunk3 for W7
            sched[(6, 7)].append(av_part(5, 0, 4, 8))
            sched[(6, 8)].append(av_part(5, 1, 4, 8))
            sched[(6, 9)].append(av_part(5, 0, 8, 12))
            sched[(6, 10)].append(av_part(5, 1, 8, 12))
            sched[(6, 11)].append(av_part(5, 0, 12, 16))
            sched[(6, 12)].append(av_part(5, 1, 12, 16))
            sched[(6, 13)].append(norm2(5))
            sched[(6, 14)].append(av_part(6, 0, 0, 4))
            sched[(6, 15)].append(av_part(6, 1, 0, 4))

            # W7 (qc3, heads 0,1): out-proj st8-11 leads, AV(6) drains,
            # AV(7) starts as its exps land
            sched[(7, 0)].append(av_part(6, 0, 4, 8))
            sched[(7, 1)].append(av_part(6, 1, 4, 8))
            sched[(7, 2)].append(out_proj_st(8))
            sched[(7, 3)].append(out_proj_st(9))
            sched[(7, 4)].append(out_proj_st(10))
            sched[(7, 5)].append(out_proj_st(11))
            sched[(7, 6)].append(av_part(6, 0, 8, 12))
            sched[(7, 7)].append(av_part(6, 1, 8, 12))
            sched[(7, 8)].append(av_part(6, 0, 12, 16))
            sched[(7, 9)].append(av_part(6, 1, 12, 16))
            sched[(7, 10)].append(norm2(6))
            sched[(7, 11)].append(av_part(7, 0, 0, 4))
            sched[(7, 12)].append(av_part(7, 1, 0, 4))
            sched[(7, 13)].append(av_part(7, 0, 4, 8))
            sched[(7, 14)].append(av_part(7, 1, 4, 8))
            sched[(7, 15)].append(av_part(7, 0, 8, 12))

            # ---------- lead-in ----------
            # K et1 + Q et1 chunk0 interleaved per hT slab: two open PSUM
            # accumulation groups so each arriving slab-pair DMA feeds both
            # projections immediately; the post-arrival chain is then just
            # 2 matmuls + 2 casts + the first score pair.
            k_ps = psum.tile([P, SC], F32, name="qk_ps", tag="mm512")
            q_ps = psum.tile([P, SC], F32, name="qk_ps", tag="mm512")
            for kt in range(NT):
                nc.tensor.matmul(
                    out=k_ps[:, :], lhsT=wk_sb[:, 1, kt, :],
                    rhs=hT_sb[:, 0, kt, :], start=(kt == 0),
                    stop=(kt == NT - 1), skip_group_check=True)
                nc.tensor.matmul(
                    out=q_ps[:, :], lhsT=wq_sb[:, 1, kt, :],
                    rhs=hT_sb[:, 0, kt, :], start=(kt == 0),
                    stop=(kt == NT - 1), skip_group_check=True)
            nc.vector.tensor_copy(out=KT_sb[:, 1, 0:SC], in_=k_ps[:, :])
            nc.vector.tensor_scalar_add(
                QT_sb[:, 1, 0:SC], q_ps[:, :], qb_sb[:, 1:2])

            # ---------- windows ----------
            # slot items are deferred one score-pair: pair(kt+1) is
            # emitted BEFORE slot kt's work so the exp stream never queues
            # behind a stalled slot item (PE is strict FIFO)
            pending = []
            for w in range(2 * NQC):
                qc, heads = win_heads(w)
                q_sl = slice(qc * SC, (qc + 1) * SC)
                qa, ka = head_qk(heads[0])
                qb, kb = head_qk(heads[1])
                for kt in range(NKT):
                    if kt % 8 == 0:
                        pt_map[(w, kt // 8)] = ptpool.tile(
                            [P, 8, 2, SC], BF, name="PT", tag="PT", bufs=5)
                    PT = pt_map[(w, kt // 8)]
                    s_ps = psum.tile([P, 2, SC], F32, name="s_ps",
                                     tag="s_ps", bufs=2)
                    # concurrent 64-row PE tiles: head a on T0 (rows 0-63),
                    # head b on T8 (rows 64-127)
                    nc.tensor.matmul(
                        out=s_ps[:, 0, :],
                        lhsT=ka[:, kt * P:(kt + 1) * P],
                        rhs=qa[:, q_sl],
                        start=True, stop=True, tile_position=(0, 0))
                    nc.tensor.matmul(
                        out=s_ps[:, 1, :],
                        lhsT=kb[:, kt * P:(kt + 1) * P],
                        rhs=qb[:, q_sl],
                        start=True, stop=True, tile_position=(64, 0))
                    nc.scalar.activation(
                        out=PT[:, kt % 8, :, :], in_=s_ps[:, :, :],
                        func=mybir.ActivationFunctionType.Exp,
                        scale=0.125)
                    for fn in pending:
                        fn()
                    pending = sched.get((w, kt), [])
            for fn in pending:
                fn()

            # ---------- tail ----------
            av_part(7, 1, 8, 12)()
            av_part(7, 0, 12, 16)()
            norm(7, 0, tail=True)()
            av_part(7, 1, 12, 16)()
            norm(7, 1, tail=True)()
            # keep the PE HAM clock warm through the ~4us norm chain —
            # otherwise the final out-proj matmuls run at 1.2GHz
            for w in range(12):
                tw_ps = psum.tile([P, 2 * P], F32, name="tw_ps", tag="s_ps")
                nc.tensor.matmul(
                    out=tw_ps[:, :], lhsT=dummy_sb[:, 0:P],
                    rhs=dummy_sb[:, :], start=True, stop=True)
            for st, qs in zip(range(12, 16),
                              ((nc.sync, nc.gpsimd), (nc.gpsimd, nc.scalar),
                               (nc.scalar, nc.sync), (nc.sync, nc.gpsimd))):
                out_proj_st(st, yq=qs, tail=True)()

    return nc


def _shard_inputs(hidden_states, cos, sin, qkv_w, qkv_b, out_w):
    """Host-side prep: per-core partition-major bf16 shards (each SBUF
    partition's data is contiguous in DRAM so DMAs use large descriptors)."""
    hs = np.asarray(hidden_states, dtype=np.float32)
    cos = np.asarray(cos, dtype=np.float32)
    sin = np.asarray(sin, dtype=np.float32)
    qkv_w = np.asarray(qkv_w, dtype=np.float32)
    qkv_b = np.asarray(qkv_b, dtype=np.float32)
    out_w = np.asarray(out_w, dtype=np.float32)

    def bf(x):
        return np.ascontiguousarray(x).astype(_BF_NP)

    def pmaj(wmat, nt):
        # [nt*P, cols] -> partition-major [P, nt*cols]
        cols = wmat.shape[1]
        return wmat.reshape(nt, P, cols).transpose(1, 0, 2).reshape(
            P, nt * cols)

    # hT chunk-major: [c, p, t, s] -> [(c p), (t s)]
    hTc_b = []
    for b in range(B):
        hT = hs[b].T                                   # [DIM, S]
        hTc = hT.reshape(NT, P, NQC, SC).transpose(2, 1, 0, 3)
        hTc_b.append(bf(hTc.reshape(NQC * P, NT * SC)))

    def pmaj_et(wT):
        # [DIM, E] -> et-major partition-major [P, et, kt, 128]
        m = wT.reshape(NT, P, 2, P).transpose(1, 2, 0, 3)   # [P, et, kt, c]
        return m.reshape(P, 2 * NT * P)

    in_maps = []
    for core in range(NCORES):
        b, g = divmod(core, GROUPS)
        e0 = E * g
        wq = qkv_w[e0:e0 + E]
        wk = qkv_w[H * D + e0:H * D + e0 + E]
        wv = qkv_w[2 * H * D + e0:2 * H * D + e0 + E]
        qb = qkv_b[e0:e0 + E].reshape(2, P).T               # [P, et]
        kb0 = qkv_b[H * D + e0:H * D + e0 + P, None]        # [P, 1]
        woutT = out_w[:, e0:e0 + E].T                    # [256, DIM]
        if g == 0:
            c = cos[b].T
            sgn = np.where(np.arange(D) % 2 == 0, -1.0, 1.0)[:, None]
            s_ = sin[b].T * sgn.astype(np.float32)
        else:
            c = np.ones((D, S), np.float32)
            s_ = np.zeros((D, S), np.float32)
        perm = np.zeros((D, D), np.float32)
        perm[np.arange(D), np.arange(D) ^ 1] = 1.0
        in_maps.append({
            "hTc": hTc_b[b],
            "wkT": bf(pmaj_et(wk.T)),
            "wqT": bf(pmaj_et(wq.T)),
            "wvT": bf(pmaj(wv.T, NT)),
            "woutT": bf(pmaj(woutT, E // P)),
            "qb": np.ascontiguousarray(qb, dtype=np.float32),
            "kb0": np.ascontiguousarray(kb0, dtype=np.float32),
            "cos_t": bf(c),
            "sin_t": bf(s_),
            "perm": bf(perm),
        })
    return in_maps


_last_results = None


def _ensure_axon_hooks():
    """run_bass_kernel_spmd imports antenv.axon_hooks when BASS_TRACE is set;
    this image's antenv lacks that module. Provide a no-op stand-in (hook=None
    -> tracing is skipped, run proceeds) so a stray BASS_TRACE can't crash."""
    try:
        import antenv.axon_hooks  # noqa: F401
    except ImportError:
        import sys as _sys
        import types as _types
        try:
            import antenv
        except ImportError:
            return
        mod = _types.ModuleType("antenv.axon_hooks")
        _state = {"hook": None}
        mod.set_axon_ntff_profile_hook = lambda h: _state.__setitem__("hook", h)
        mod.get_axon_ntff_profile_hook = lambda: _state["hook"]
        _sys.modules["antenv.axon_hooks"] = mod
        antenv.axon_hooks = mod


def kernel(hidden_states, cos, sin, qkv_w, qkv_b, out_w, out_b):
    global _last_results
    _ensure_axon_hooks()
    in_maps = _shard_inputs(hidden_states, cos, sin, qkv_w, qkv_b, out_w)
    nc = _build_nc()
    nc.compile()  # Bacc defers register allocation to compile()
    res = run_bass_kernel_spmd(nc, in_maps, core_ids=list(range(NCORES)))
    _last_results = res
    ys = [np.asarray(res.results[c]["y"], dtype=np.float32) for c in range(NCORES)]
    # host-folded V bias: attn_out = OT/den + b_v, so y gains out_w @ b_v
    ow = np.asarray(out_w, dtype=np.float32)
    bv = np.asarray(qkv_b, dtype=np.float32)[2 * H * D:]
    const = np.asarray(out_b, dtype=np.float32) + ow @ bv
    out = np.stack([
        ys[0] + ys[1] + ys[2] + ys[3] + const[None, :],
        ys[4] + ys[5] + ys[6] + ys[7] + const[None, :],
    ])
    return out.astype(np.float32)


if __name__ == "__main__":
    nc = _build_nc()
    n_inst = sum(len(bb.instructions) for f in nc.m.functions for bb in f.blocks)
    print(f"built nc with {n_inst} instructions")

